# revision 1
# baseline (speedup 1.0000x reference)
"""DrBC GNN message-passing kernel for 8 Trainium2 NeuronCores.

Sharding: nodes split contiguously across 8 cores. Edges bucketed by target
node's owner core and 128-node target tile. Per layer each core:
  1. dma_gather's source-node rows (bf16) from a replicated node table in HBM
     (split into lo/hi halves so indices fit int16),
  2. scatter-adds them into per-tile aggregates via PE matmuls against
     on-the-fly one-hot selection matrices (edge norm folded in),
  3. runs the GRU update (bias-opener matmuls + fused gi+gh accumulation),
  4. l2-normalizes, updates the running layer-max, and
  5. AllGathers the new shard into the next layer's replicated table
     (skipped after the last layer).
Decoder (2-layer MLP on the layer-max) runs on the local shard only.
"""

import numpy as np
import ml_dtypes

import concourse.bass as bass
import concourse.bacc as bacc
import concourse.mybir as mybir
from concourse.tile import TileContext
from concourse.bass_utils import run_bass_kernel_spmd

F32 = mybir.dt.float32
BF16 = mybir.dt.bfloat16
I16 = mybir.dt.int16
AF = mybir.ActivationFunctionType
ALU = mybir.AluOpType

EPS = 1e-12


class Cfg:
    def __init__(self, N=50000, E=600000, L=5, n_cores=8, group_tiles=2,
                 single_packet=False):
        self.N, self.E, self.L, self.n_cores = N, E, L, n_cores
        self.H = 128
        self.IN = 3
        self.HID = 64
        assert N % n_cores == 0
        self.NSH = N // n_cores              # nodes per core
        self.NT = (self.NSH + 127) // 128    # node tiles per core
        self.NSH_PAD = self.NT * 128
        self.SPLIT = (N // 2 + 127) // 128 * 128  # lo/hi table split (int16 idx)
        assert self.SPLIT < 32768 and N - self.SPLIT < 32768
        self.GT = group_tiles                # tiles per gather group
        self.single_packet = single_packet


def build_plan(cfg, edge_idx):
    """Bucket edges by (core, target tile), pad chunk counts uniformly across
    cores (SPMD: one NEFF), build per-core gather-index/chunk-meta arrays."""
    row = np.asarray(edge_idx[0], dtype=np.int64)
    col = np.asarray(edge_idx[1], dtype=np.int64)
    N = cfg.N
    deg = np.bincount(col, minlength=N).astype(np.float64) + 1.0
    dinv = deg ** -0.5
    norm = (dinv[row] * dinv[col]).astype(np.float32)

    n_cores, NT = cfg.n_cores, cfg.NT
    core_of = col // cfg.NSH
    tile_of = (col % cfg.NSH) // 128
    v_of = ((col % cfg.NSH) % 128).astype(np.float32)
    is_hi = (row >= cfg.SPLIT).astype(np.int64)
    rloc = np.where(is_hi == 1, row - cfg.SPLIT, row).astype(np.int16)

    # bucket key: (core, tile, half)
    key = (core_of * NT + tile_of) * 2 + is_hi
    nkey = n_cores * NT * 2
    order = np.argsort(key, kind="stable")
    ks, rs, vs, ns = key[order], rloc[order], v_of[order], norm[order]
    counts = np.bincount(ks, minlength=nkey)
    starts = np.concatenate([[0], np.cumsum(counts)[:-1]])

    cnt = counts.reshape(n_cores, NT, 2)
    # uniform per-tile chunk counts = max over cores (>=1 so psum groups exist)
    nlo = np.maximum((cnt[:, :, 0].max(axis=0) + 127) // 128, 1)
    nhi = np.maximum((cnt[:, :, 1].max(axis=0) + 127) // 128, 1)
    nch2 = np.stack([nlo, nhi], axis=1)          # [NT, 2]

    groups = [list(range(t0, min(t0 + cfg.GT, NT))) for t0 in range(0, NT, cfg.GT)]
    NCHUNK = int(nlo.sum() + nhi.sum())
    W_tot = 8 * NCHUNK

    # padded flat layout, order (t, half) shared by all cores
    pad_off = np.zeros((NT, 2), dtype=np.int64)
    po = 0
    for t in range(NT):
        for h in (0, 1):
            pad_off[t, h] = po
            po += int(nch2[t, h]) * 128
    PADTOT = po

    # scatter edges into per-core padded arrays
    idxpad = np.zeros((n_cores, PADTOT), dtype=np.int16)
    vpad = np.zeros((n_cores, PADTOT), dtype=np.float32)
    npad = np.zeros((n_cores, PADTOT), dtype=np.float32)
    rank = np.arange(len(ks)) - starts[ks]
    kc, kt, kh = ks // (NT * 2), (ks // 2) % NT, ks % 2
    dest = pad_off[kt, kh] + rank
    idxpad[kc, dest] = rs
    vpad[kc, dest] = vs
    npad[kc, dest] = ns

    # chunk ids in emission order: per group, per tile: lo chunks then hi chunks
    chunk_id = {}
    cid = 0
    for ts in groups:
        for t in ts:
            for j in range(int(nlo[t])):
                chunk_id[(t, 0, j)] = cid; cid += 1
            for j in range(int(nhi[t])):
                chunk_id[(t, 1, j)] = cid; cid += 1
    assert cid == NCHUNK

    # gather-call layout: per group, one lo call then one hi call
    call_off = []
    buf_pos = {}
    col_cursor = 0
    for ts in groups:
        lo_n = int(sum(nlo[t] for t in ts))
        hi_n = int(sum(nhi[t] for t in ts))
        call_off.append((col_cursor, lo_n, col_cursor + 8 * lo_n, hi_n))
        col_cursor += 8 * (lo_n + hi_n)
        p = 0
        for t in ts:
            for j in range(int(nlo[t])):
                buf_pos[(t, 0, j)] = p; p += 1
        p = 0
        for t in ts:
            for j in range(int(nhi[t])):
                buf_pos[(t, 1, j)] = p; p += 1
    assert col_cursor == W_tot

    gidx = np.zeros((n_cores, 128, W_tot), dtype=np.int16)
    cmeta = np.zeros((n_cores, 128, 2 * NCHUNK), dtype=np.float32)
    for c in range(n_cores):
        for g_, ts in enumerate(groups):
            lo_off, lo_n, hi_off, hi_n = call_off[g_]
            for half, off, nch in ((0, lo_off, nlo), (1, hi_off, nhi)):
                parts = [idxpad[c, pad_off[t, half]:
                                pad_off[t, half] + int(nch[t]) * 128]
                         for t in ts]
                idx_flat = np.concatenate(parts)
                wrapped = idx_flat.reshape(-1, 16).T
                gidx[c, :, off:off + wrapped.shape[1]] = np.tile(wrapped, (8, 1))
        for t in range(NT):
            for half, nch in ((0, nlo), (1, nhi)):
                c0 = chunk_id[(t, half, 0)]
                n_ = int(nch[t])
                sl = slice(pad_off[t, half], pad_off[t, half] + n_ * 128)
                cmeta[c, :, c0:c0 + n_] = vpad[c, sl].reshape(n_, 128).T
                cmeta[c, :, NCHUNK + c0:NCHUNK + c0 + n_] = \
                    npad[c, sl].reshape(n_, 128).T

    return dict(nlo=nlo, nhi=nhi, groups=groups, NCHUNK=NCHUNK, W_tot=W_tot,
                call_off=call_off, chunk_id=chunk_id, buf_pos=buf_pos), gidx, cmeta


def build_nc(cfg, plan, b_out_val):
    nc = bacc.Bacc("TRN2", target_bir_lowering=False, debug=False,
                   num_devices=cfg.n_cores)
    H, NT, NSH, L = cfg.H, cfg.NT, cfg.NSH, cfg.L
    NCHUNK = plan["NCHUNK"]
    nlo, nhi, groups = plan["nlo"], plan["nhi"], plan["groups"]
    call_off, chunk_id, buf_pos = plan["call_off"], plan["chunk_id"], plan["buf_pos"]
    RG = [list(range(cfg.n_cores))]
    last_rows = NSH - (NT - 1) * 128

    p_gidx = nc.declare_dram_parameter("gidx", [128, plan["W_tot"]], I16, isOutput=False)
    p_cmeta = nc.declare_dram_parameter("cmeta", [128, 2 * NCHUNK], F32, isOutput=False)
    p_xT = nc.declare_dram_parameter("xT", [cfg.IN, cfg.NSH_PAD], F32, isOutput=False)
    p_iota = nc.declare_dram_parameter("iota", [128, 128], F32, isOutput=False)
    p_ident = nc.declare_dram_parameter("ident", [128, 128], BF16, isOutput=False)
    p_wemb = nc.declare_dram_parameter("wembT", [cfg.IN, H], F32, isOutput=False)
    p_bemb = nc.declare_dram_parameter("bemb", [1, H], F32, isOutput=False)
    p_wih_rz = nc.declare_dram_parameter("wih_rz", [H, L * 256], BF16, isOutput=False)
    p_wih_n = nc.declare_dram_parameter("wih_n", [H, L * 128], BF16, isOutput=False)
    p_whh_rz = nc.declare_dram_parameter("whh_rz", [H, L * 256], BF16, isOutput=False)
    p_whh_n = nc.declare_dram_parameter("whh_n", [H, L * 128], BF16, isOutput=False)
    p_ball = nc.declare_dram_parameter("ball", [1, L * 512], F32, isOutput=False)
    p_whid = nc.declare_dram_parameter("whidT", [H, cfg.HID], BF16, isOutput=False)
    p_bhid = nc.declare_dram_parameter("bhid", [cfg.HID, 1], F32, isOutput=False)
    p_wout = nc.declare_dram_parameter("woutT", [cfg.HID, 1], BF16, isOutput=False)
    p_ones = nc.declare_dram_parameter("ones1", [1, 128], F32, isOutput=False)
    p_out = nc.declare_dram_parameter("out", [NSH, 1], F32, isOutput=True)

    tabA = nc.dram_tensor("tabA", [cfg.N, H], BF16, addr_space="Shared")
    tabB = nc.dram_tensor("tabB", [cfg.N, H], BF16, addr_space="Shared")
    shard_out = nc.dram_tensor("shard_out", [NSH, H], BF16)

    with TileContext(nc) as tc:
        with (
            tc.tile_pool(name="consts", bufs=1) as cpool,
            tc.tile_pool(name="zpool", bufs=1) as zpool,
            tc.tile_pool(name="gbuf", bufs=2) as gpool,
            tc.tile_pool(name="work", bufs=3) as wpool,
            tc.tile_pool(name="spool", bufs=4) as spool,
            tc.tile_pool(name="psA", bufs=2, space="PSUM") as ppool,
            tc.tile_pool(name="psB", bufs=2, space="PSUM") as qpool,
            tc.tile_pool(name="psT", bufs=2, space="PSUM") as tpool,
        ):
            def load_const(ap, dtype, tag):
                t = cpool.tile(list(ap.shape), dtype, tag=tag)
                nc.sync.dma_start(out=t[:], in_=ap[:])
                return t

            gidx_t = load_const(p_gidx, I16, "gidx")
            cmeta_t = load_const(p_cmeta, F32, "cmeta")
            xT_t = load_const(p_xT, F32, "xT")
            iota_t = load_const(p_iota, F32, "iota")
            ident_t = load_const(p_ident, BF16, "ident")
            wemb_t = load_const(p_wemb, F32, "wemb")
            bemb_t = load_const(p_bemb, F32, "bemb")
            wih_rz_t = load_const(p_wih_rz, BF16, "wihrz")
            wih_n_t = load_const(p_wih_n, BF16, "wihn")
            whh_rz_t = load_const(p_whh_rz, BF16, "whhrz")
            whh_n_t = load_const(p_whh_n, BF16, "whhn")
            ball_t = load_const(p_ball, F32, "ball")
            whid_t = load_const(p_whid, BF16, "whid")
            bhid_t = load_const(p_bhid, F32, "bhid")
            wout_t = load_const(p_wout, BF16, "wout")
            ones_t = load_const(p_ones, F32, "ones")

            zmax_t = zpool.tile([128, cfg.NSH_PAD], BF16, tag="zmax")
            h_shA = zpool.tile([128, cfg.NSH_PAD], BF16, tag="hshA")
            h_shB = zpool.tile([128, cfg.NSH_PAD], BF16, tag="hshB")
            h_bufs = [h_shA, h_shB]

            def rows_of(t):
                return 128 if t < NT - 1 else last_rows

            def l2norm_bf(pre_t, out_ap):
                """l2-normalize pre_t [128,H] f32 -> bf16 out_ap."""
                sq = wpool.tile([128, H], F32, tag="sq")
                ss = wpool.tile([128, 1], F32, tag="ss")
                nc.scalar.activation(sq[:], pre_t[:], AF.Square, accum_out=ss[:])
                s1 = wpool.tile([128, 1], F32, tag="s1")
                nc.scalar.activation(s1[:], ss[:], AF.Sqrt)
                s2 = wpool.tile([128, 1], F32, tag="s2")
                nc.vector.tensor_scalar_max(s2[:], s1[:], EPS)
                rec = wpool.tile([128, 1], F32, tag="rec")
                nc.vector.reciprocal(rec[:], s2[:])
                nc.vector.tensor_scalar_mul(out_ap, pre_t[:], rec[:])

            # ================= EMBED =================
            for t in range(NT):
                ps = ppool.tile([128, H], F32, tag="acc")
                nc.tensor.matmul(ps[:], ones_t[:], bemb_t[:], start=True, stop=False)
                nc.tensor.matmul(ps[:], xT_t[:, t * 128:(t + 1) * 128], wemb_t[:],
                                 start=False, stop=True)
                h0f = wpool.tile([128, H], F32, tag="pref")
                nc.scalar.activation(h0f[:], ps[:], AF.Relu)
                hsl = h_bufs[0][:, t * 128:(t + 1) * 128]
                l2norm_bf(h0f, hsl)
                nc.vector.tensor_copy(zmax_t[:, t * 128:(t + 1) * 128], hsl)
                r = rows_of(t)
                nc.sync.dma_start(out=shard_out[t * 128: t * 128 + r, :],
                                  in_=h_bufs[0][:r, t * 128: t * 128 + H])
            nc.gpsimd.collective_compute(
                "AllGather", ALU.bypass, replica_groups=RG,
                ins=[shard_out[:]], outs=[tabA[:]],
            )

            # ================= GRU LAYERS =================
            for l in range(L):
                tab_prev = tabA if l % 2 == 0 else tabB
                tab_cur = tabB if l % 2 == 0 else tabA
                for g_, ts in enumerate(groups):
                    lo_off, lo_n, hi_off, hi_n = call_off[g_]
                    glo = gpool.tile([128, lo_n * 128], BF16, tag="glo")
                    ghi = gpool.tile([128, hi_n * 128], BF16, tag="ghi")
                    nc.gpsimd.dma_gather(
                        glo[:].rearrange("p (c e) -> p c e", e=128),
                        tab_prev[0:cfg.SPLIT, :],
                        gidx_t[:, lo_off: lo_off + 8 * lo_n],
                        lo_n * 128, lo_n * 128, H,
                        single_packet=cfg.single_packet,
                    )
                    nc.gpsimd.dma_gather(
                        ghi[:].rearrange("p (c e) -> p c e", e=128),
                        tab_prev[cfg.SPLIT: cfg.N, :],
                        gidx_t[:, hi_off: hi_off + 8 * hi_n],
                        hi_n * 128, hi_n * 128, H,
                        single_packet=cfg.single_packet,
                    )
                    for t in ts:
                        r = rows_of(t)
                        base = t * 128
                        h_prev = h_bufs[l % 2]
                        h_next = h_bufs[(l + 1) % 2]
                        hp_sl = h_prev[:, base:base + H]
                        tps = tpool.tile([128, H], BF16, tag="tps")
                        nc.tensor.transpose(tps[:], hp_sl, ident_t[:])
                        hT = wpool.tile([128, H], BF16, tag="hT")
                        nc.scalar.activation(hT[:], tps[:], AF.Copy)

                        # ---- wide one-hot S for all chunks of this tile ----
                        nch = int(nlo[t] + nhi[t])
                        cid0 = chunk_id[(t, 0, 0)]
                        S = spool.tile([128, nch * 128], BF16, tag="S")
                        S3 = S[:].rearrange("p (c e) -> p c e", e=128)
                        iota_b = iota_t[:].unsqueeze(1).broadcast_to((128, nch, 128))
                        col_b = cmeta_t[:, cid0:cid0 + nch].unsqueeze(2) \
                            .broadcast_to((128, nch, 128))
                        nrm_b = cmeta_t[:, NCHUNK + cid0:NCHUNK + cid0 + nch] \
                            .unsqueeze(2).broadcast_to((128, nch, 128))
                        nc.vector.tensor_tensor(S3, iota_b, col_b, ALU.is_equal)
                        nc.vector.tensor_tensor(S3, S3, nrm_b, ALU.mult)

                        # ---- scatter: aggT[H, v] += G_chunk^T @ S_chunk ----
                        aggT_ps = ppool.tile([128, H], F32, tag="acc")
                        for k in range(nch):
                            if k < int(nlo[t]):
                                buf, bp = glo, buf_pos[(t, 0, k)]
                            else:
                                buf, bp = ghi, buf_pos[(t, 1, k - int(nlo[t]))]
                            nc.tensor.matmul(
                                aggT_ps[:],
                                buf[:, bp * 128:(bp + 1) * 128],
                                S[:, k * 128:(k + 1) * 128],
                                start=(k == 0), stop=(k == nch - 1),
                            )
                        aggT = wpool.tile([128, H], BF16, tag="aggT")
                        nc.scalar.activation(aggT[:], aggT_ps[:], AF.Copy)

                        # ---- GRU gate matmuls (fused bias openers) ----
                        grz = qpool.tile([128, 256], F32, tag="grz")
                        gn = qpool.tile([128, 256], F32, tag="gn")
                        nc.tensor.matmul(grz[:], ones_t[:],
                                         ball_t[:, l * 512:l * 512 + 256],
                                         start=True, stop=False)
                        nc.tensor.matmul(grz[:], aggT[:],
                                         wih_rz_t[:, l * 256:(l + 1) * 256],
                                         start=False, stop=False)
                        nc.tensor.matmul(grz[:], hT[:],
                                         whh_rz_t[:, l * 256:(l + 1) * 256],
                                         start=False, stop=True)
                        nc.tensor.matmul(gn[:, 0:128], ones_t[:],
                                         ball_t[:, l * 512 + 256:l * 512 + 384],
                                         start=True, stop=False)
                        nc.tensor.matmul(gn[:, 0:128], aggT[:],
                                         wih_n_t[:, l * 128:(l + 1) * 128],
                                         start=False, stop=True)
                        nc.tensor.matmul(gn[:, 128:256], ones_t[:],
                                         ball_t[:, l * 512 + 384:(l + 1) * 512],
                                         start=True, stop=False)
                        nc.tensor.matmul(gn[:, 128:256], hT[:],
                                         whh_n_t[:, l * 128:(l + 1) * 128],
                                         start=False, stop=True)

                        # ---- gates ----
                        rzt = wpool.tile([128, 256], F32, tag="rzt")
                        nc.scalar.activation(rzt[:], grz[:], AF.Sigmoid)
                        t1 = wpool.tile([128, H], F32, tag="t1")
                        nc.vector.tensor_mul(t1[:], rzt[:, 0:128], gn[:, 128:256])
                        t2 = wpool.tile([128, H], F32, tag="t2")
                        nc.vector.tensor_add(t2[:], t1[:], gn[:, 0:128])
                        ng = wpool.tile([128, H], F32, tag="ng")
                        nc.scalar.activation(ng[:], t2[:], AF.Tanh)
                        d = wpool.tile([128, H], F32, tag="d")
                        nc.vector.tensor_sub(d[:], hp_sl, ng[:])
                        e = wpool.tile([128, H], F32, tag="e")
                        nc.vector.tensor_mul(e[:], d[:], rzt[:, 128:256])
                        pre = wpool.tile([128, H], F32, tag="pref")
                        nc.vector.tensor_add(pre[:], e[:], ng[:])
                        hn_sl = h_next[:, base:base + H]
                        l2norm_bf(pre, hn_sl)
                        nc.vector.tensor_max(zmax_t[:, base:base + 128],
                                             zmax_t[:, base:base + 128], hn_sl)
                        if l < L - 1:
                            nc.sync.dma_start(out=shard_out[base: base + r, :],
                                              in_=h_next[:r, base:base + H])
                if l < L - 1:
                    nc.gpsimd.collective_compute(
                        "AllGather", ALU.bypass, replica_groups=RG,
                        ins=[shard_out[:]], outs=[tab_cur[:]],
                    )

            # ================= DECODER =================
            for t in range(NT):
                r = rows_of(t)
                base = t * 128
                tps = tpool.tile([128, H], BF16, tag="tps")
                nc.tensor.transpose(tps[:], zmax_t[:, base:base + 128], ident_t[:])
                zT = wpool.tile([128, H], BF16, tag="zT")
                nc.scalar.activation(zT[:], tps[:], AF.Copy)
                hid_ps = qpool.tile([cfg.HID, 128], F32, tag="grz")
                nc.tensor.matmul(hid_ps[:], whid_t[:], zT[:], start=True, stop=True)
                hid = wpool.tile([cfg.HID, 128], BF16, tag="hid")
                nc.scalar.activation(hid[:], hid_ps[:], AF.Relu, bias=bhid_t[:])
                o_ps = ppool.tile([1, 128], F32, tag="acc")
                nc.tensor.matmul(o_ps[:], wout_t[:], hid[:], start=True, stop=True)
                o_sb = wpool.tile([1, 128], F32, tag="osb")
                nc.scalar.activation(o_sb[:], o_ps[:], AF.Copy, bias=float(b_out_val))
                nc.sync.dma_start(out=p_out[base: base + r, :], in_=o_sb[:1, :r])
    nc.compile()
    return nc


def make_in_maps(cfg, inputs, plan, gidx, cmeta):
    bf = ml_dtypes.bfloat16
    L, H, NSH = cfg.L, cfg.H, cfg.NSH
    x = np.asarray(inputs["x"], np.float32)
    w_ih = np.asarray(inputs["w_ih"], np.float32)
    w_hh = np.asarray(inputs["w_hh"], np.float32)
    b_ih = np.asarray(inputs["b_ih"], np.float32)
    b_hh = np.asarray(inputs["b_hh"], np.float32)

    wih_rz = np.concatenate([w_ih[l, :256, :].T for l in range(L)], axis=1)
    wih_n = np.concatenate([w_ih[l, 256:384, :].T for l in range(L)], axis=1)
    whh_rz = np.concatenate([w_hh[l, :256, :].T for l in range(L)], axis=1)
    whh_n = np.concatenate([w_hh[l, 256:384, :].T for l in range(L)], axis=1)
    ball = np.concatenate(
        [np.concatenate([b_ih[l, :256] + b_hh[l, :256],
                         b_ih[l, 256:384], b_hh[l, 256:384]])
         for l in range(L)])[None, :]

    common = {
        "iota": np.tile(np.arange(128, dtype=np.float32), (128, 1)),
        "ident": np.eye(128, dtype=bf),
        "wembT": np.ascontiguousarray(np.asarray(inputs["W_embed"], np.float32).T),
        "bemb": np.asarray(inputs["b_embed"], np.float32)[None, :],
        "wih_rz": np.ascontiguousarray(wih_rz, dtype=bf),
        "wih_n": np.ascontiguousarray(wih_n, dtype=bf),
        "whh_rz": np.ascontiguousarray(whh_rz, dtype=bf),
        "whh_n": np.ascontiguousarray(whh_n, dtype=bf),
        "ball": np.ascontiguousarray(ball),
        "whidT": np.ascontiguousarray(np.asarray(inputs["W_hid"], np.float32).T,
                                      dtype=bf),
        "bhid": np.asarray(inputs["b_hid"], np.float32)[:, None],
        "woutT": np.ascontiguousarray(np.asarray(inputs["W_out"], np.float32).T,
                                      dtype=bf),
        "ones1": np.ones((1, 128), np.float32),
    }
    in_maps = []
    for c in range(cfg.n_cores):
        xT = np.zeros((cfg.IN, cfg.NSH_PAD), np.float32)
        xT[:, :NSH] = x[c * NSH:(c + 1) * NSH, :].T
        m = dict(common)
        m["xT"] = xT
        m["gidx"] = gidx[c]
        m["cmeta"] = cmeta[c]
        in_maps.append(m)
    return in_maps


def kernel(**inputs):
    cfg = Cfg()
    plan, gidx, cmeta = build_plan(cfg, np.asarray(inputs["edge_idx"]))
    nc = build_nc(cfg, plan, float(np.asarray(inputs["b_out"]).ravel()[0]))
    in_maps = make_in_maps(cfg, inputs, plan, gidx, cmeta)
    res = run_bass_kernel_spmd(nc, in_maps, list(range(cfg.n_cores)))
    out = np.concatenate([res.results[c]["out"] for c in range(cfg.n_cores)], axis=0)
    return out.astype(np.float32)



# revision 3
# speedup vs baseline: 1.3541x; 1.3541x over previous
"""DrBC GNN message-passing kernel for 8 Trainium2 NeuronCores.

Sharding: nodes split contiguously across 8 cores. Edges bucketed by target
node's owner core and 128-node target tile. Per layer each core:
  1. dma_gather's source-node rows (bf16) from a replicated node table in HBM
     (split into lo/hi halves so indices fit int16),
  2. scatter-adds them into per-tile aggregates via PE matmuls against
     on-the-fly one-hot selection matrices (edge norm folded in),
  3. runs the GRU update (bias-opener matmuls + fused gi+gh accumulation),
  4. l2-normalizes, updates the running layer-max, and
  5. AllGathers the new shard into the next layer's replicated table
     (skipped after the last layer).
Decoder (2-layer MLP on the layer-max) runs on the local shard only.
"""

import numpy as np
import ml_dtypes

import concourse.bass as bass
import concourse.bacc as bacc
import concourse.mybir as mybir
from concourse.tile import TileContext
from concourse.bass_utils import run_bass_kernel_spmd

F32 = mybir.dt.float32
BF16 = mybir.dt.bfloat16
I16 = mybir.dt.int16
AF = mybir.ActivationFunctionType
ALU = mybir.AluOpType

EPS = 1e-12


class Cfg:
    def __init__(self, N=50000, E=600000, L=5, n_cores=8, group_tiles=2,
                 single_packet=False):
        self.N, self.E, self.L, self.n_cores = N, E, L, n_cores
        self.H = 128
        self.IN = 3
        self.HID = 64
        assert N % n_cores == 0
        self.NSH = N // n_cores              # nodes per core
        self.NT = (self.NSH + 127) // 128    # node tiles per core
        self.NSH_PAD = self.NT * 128
        self.SPLIT = (N // 2 + 127) // 128 * 128  # lo/hi table split (int16 idx)
        assert self.SPLIT < 32768 and N - self.SPLIT < 32768
        self.GT = group_tiles                # tiles per gather group
        self.single_packet = single_packet


def build_plan(cfg, edge_idx):
    """Bucket edges by (core, target tile), pad chunk counts uniformly across
    cores (SPMD: one NEFF), build per-core gather-index/chunk-meta arrays."""
    row = np.asarray(edge_idx[0], dtype=np.int64)
    col = np.asarray(edge_idx[1], dtype=np.int64)
    N = cfg.N
    deg = np.bincount(col, minlength=N).astype(np.float64) + 1.0
    dinv = deg ** -0.5
    norm = (dinv[row] * dinv[col]).astype(np.float32)

    n_cores, NT = cfg.n_cores, cfg.NT
    core_of = col // cfg.NSH
    tile_of = (col % cfg.NSH) // 128
    v_of = ((col % cfg.NSH) % 128).astype(np.float32)
    is_hi = (row >= cfg.SPLIT).astype(np.int64)
    rloc = np.where(is_hi == 1, row - cfg.SPLIT, row).astype(np.int16)

    # bucket key: (core, tile, half)
    key = (core_of * NT + tile_of) * 2 + is_hi
    nkey = n_cores * NT * 2
    order = np.argsort(key, kind="stable")
    ks, rs, vs, ns = key[order], rloc[order], v_of[order], norm[order]
    counts = np.bincount(ks, minlength=nkey)
    starts = np.concatenate([[0], np.cumsum(counts)[:-1]])

    cnt = counts.reshape(n_cores, NT, 2)
    # uniform per-tile chunk counts = max over cores (>=1 so psum groups exist)
    nlo = np.maximum((cnt[:, :, 0].max(axis=0) + 127) // 128, 1)
    nhi = np.maximum((cnt[:, :, 1].max(axis=0) + 127) // 128, 1)
    nch2 = np.stack([nlo, nhi], axis=1)          # [NT, 2]

    groups = [list(range(t0, min(t0 + cfg.GT, NT))) for t0 in range(0, NT, cfg.GT)]
    NCHUNK = int(nlo.sum() + nhi.sum())
    W_tot = 8 * NCHUNK

    # padded flat layout, order (t, half) shared by all cores
    pad_off = np.zeros((NT, 2), dtype=np.int64)
    po = 0
    for t in range(NT):
        for h in (0, 1):
            pad_off[t, h] = po
            po += int(nch2[t, h]) * 128
    PADTOT = po

    # scatter edges into per-core padded arrays
    idxpad = np.zeros((n_cores, PADTOT), dtype=np.int16)
    vpad = np.zeros((n_cores, PADTOT), dtype=np.float32)
    npad = np.zeros((n_cores, PADTOT), dtype=np.float32)
    rank = np.arange(len(ks)) - starts[ks]
    kc, kt, kh = ks // (NT * 2), (ks // 2) % NT, ks % 2
    dest = pad_off[kt, kh] + rank
    idxpad[kc, dest] = rs
    vpad[kc, dest] = vs
    npad[kc, dest] = ns

    # chunk ids in emission order: per group, per tile: lo chunks then hi chunks
    chunk_id = {}
    cid = 0
    for ts in groups:
        for t in ts:
            for j in range(int(nlo[t])):
                chunk_id[(t, 0, j)] = cid; cid += 1
            for j in range(int(nhi[t])):
                chunk_id[(t, 1, j)] = cid; cid += 1
    assert cid == NCHUNK

    # gather-call layout: per group, one lo call then one hi call
    call_off = []
    buf_pos = {}
    col_cursor = 0
    for ts in groups:
        lo_n = int(sum(nlo[t] for t in ts))
        hi_n = int(sum(nhi[t] for t in ts))
        call_off.append((col_cursor, lo_n, col_cursor + 8 * lo_n, hi_n))
        col_cursor += 8 * (lo_n + hi_n)
        p = 0
        for t in ts:
            for j in range(int(nlo[t])):
                buf_pos[(t, 0, j)] = p; p += 1
        p = 0
        for t in ts:
            for j in range(int(nhi[t])):
                buf_pos[(t, 1, j)] = p; p += 1
    assert col_cursor == W_tot

    gidx = np.zeros((n_cores, 128, W_tot), dtype=np.int16)
    cmeta = np.zeros((n_cores, 128, 2 * NCHUNK), dtype=np.float32)
    for c in range(n_cores):
        for g_, ts in enumerate(groups):
            lo_off, lo_n, hi_off, hi_n = call_off[g_]
            for half, off, nch in ((0, lo_off, nlo), (1, hi_off, nhi)):
                parts = [idxpad[c, pad_off[t, half]:
                                pad_off[t, half] + int(nch[t]) * 128]
                         for t in ts]
                idx_flat = np.concatenate(parts)
                wrapped = idx_flat.reshape(-1, 16).T
                gidx[c, :, off:off + wrapped.shape[1]] = np.tile(wrapped, (8, 1))
        for t in range(NT):
            for half, nch in ((0, nlo), (1, nhi)):
                c0 = chunk_id[(t, half, 0)]
                n_ = int(nch[t])
                sl = slice(pad_off[t, half], pad_off[t, half] + n_ * 128)
                cmeta[c, :, c0:c0 + n_] = vpad[c, sl].reshape(n_, 128).T
                cmeta[c, :, NCHUNK + c0:NCHUNK + c0 + n_] = \
                    npad[c, sl].reshape(n_, 128).T

    return dict(nlo=nlo, nhi=nhi, groups=groups, NCHUNK=NCHUNK, W_tot=W_tot,
                call_off=call_off, chunk_id=chunk_id, buf_pos=buf_pos), gidx, cmeta


def build_nc(cfg, plan, b_out_val):
    nc = bacc.Bacc("TRN2", target_bir_lowering=False, debug=False,
                   num_devices=cfg.n_cores, num_swdge_queues=4,
                   dynamic_dma_scratch_size=32768)
    H, NT, NSH, L = cfg.H, cfg.NT, cfg.NSH, cfg.L
    NCHUNK = plan["NCHUNK"]
    nlo, nhi, groups = plan["nlo"], plan["nhi"], plan["groups"]
    call_off, chunk_id, buf_pos = plan["call_off"], plan["chunk_id"], plan["buf_pos"]
    RG = [list(range(cfg.n_cores))]
    last_rows = NSH - (NT - 1) * 128

    p_gidx = nc.declare_dram_parameter("gidx", [128, plan["W_tot"]], I16, isOutput=False)
    p_cmeta = nc.declare_dram_parameter("cmeta", [128, 2 * NCHUNK], F32, isOutput=False)
    p_xT = nc.declare_dram_parameter("xT", [cfg.IN, cfg.NSH_PAD], F32, isOutput=False)
    p_iota = nc.declare_dram_parameter("iota", [128, 128], F32, isOutput=False)
    p_ident = nc.declare_dram_parameter("ident", [128, 128], BF16, isOutput=False)
    p_wemb = nc.declare_dram_parameter("wembT", [cfg.IN, H], F32, isOutput=False)
    p_bemb = nc.declare_dram_parameter("bemb", [1, H], F32, isOutput=False)
    p_wih_rz = nc.declare_dram_parameter("wih_rz", [H, L * 256], BF16, isOutput=False)
    p_wih_n = nc.declare_dram_parameter("wih_n", [H, L * 128], BF16, isOutput=False)
    p_whh_rz = nc.declare_dram_parameter("whh_rz", [H, L * 256], BF16, isOutput=False)
    p_whh_n = nc.declare_dram_parameter("whh_n", [H, L * 128], BF16, isOutput=False)
    p_ball = nc.declare_dram_parameter("ball", [1, L * 512], F32, isOutput=False)
    p_whid = nc.declare_dram_parameter("whidT", [H, cfg.HID], BF16, isOutput=False)
    p_bhid = nc.declare_dram_parameter("bhid", [cfg.HID, 1], F32, isOutput=False)
    p_wout = nc.declare_dram_parameter("woutT", [cfg.HID, 1], BF16, isOutput=False)
    p_ones = nc.declare_dram_parameter("ones1", [1, 128], F32, isOutput=False)
    p_out = nc.declare_dram_parameter("out", [NSH, 1], F32, isOutput=True)

    tabA = nc.dram_tensor("tabA", [cfg.N, H], BF16, addr_space="Shared")
    tabB = nc.dram_tensor("tabB", [cfg.N, H], BF16, addr_space="Shared")
    shard_out = nc.dram_tensor("shard_out", [NSH, H], BF16)

    with TileContext(nc) as tc:
        with (
            tc.tile_pool(name="consts", bufs=1) as cpool,
            tc.tile_pool(name="zpool", bufs=1) as zpool,
            tc.tile_pool(name="gbuf", bufs=2) as gpool,
            tc.tile_pool(name="work", bufs=3) as wpool,
            tc.tile_pool(name="spool", bufs=4) as spool,
            tc.tile_pool(name="psA", bufs=2, space="PSUM") as ppool,
            tc.tile_pool(name="psB", bufs=2, space="PSUM") as qpool,
            tc.tile_pool(name="psT", bufs=2, space="PSUM") as tpool,
        ):
            def load_const(ap, dtype, tag):
                t = cpool.tile(list(ap.shape), dtype, tag=tag)
                nc.sync.dma_start(out=t[:], in_=ap[:])
                return t

            gidx_t = load_const(p_gidx, I16, "gidx")
            cmeta_t = load_const(p_cmeta, F32, "cmeta")
            xT_t = load_const(p_xT, F32, "xT")
            iota_t = load_const(p_iota, F32, "iota")
            ident_t = load_const(p_ident, BF16, "ident")
            wemb_t = load_const(p_wemb, F32, "wemb")
            bemb_t = load_const(p_bemb, F32, "bemb")
            wih_rz_t = load_const(p_wih_rz, BF16, "wihrz")
            wih_n_t = load_const(p_wih_n, BF16, "wihn")
            whh_rz_t = load_const(p_whh_rz, BF16, "whhrz")
            whh_n_t = load_const(p_whh_n, BF16, "whhn")
            ball_t = load_const(p_ball, F32, "ball")
            whid_t = load_const(p_whid, BF16, "whid")
            bhid_t = load_const(p_bhid, F32, "bhid")
            wout_t = load_const(p_wout, BF16, "wout")
            ones_t = load_const(p_ones, F32, "ones")

            zmax_t = zpool.tile([128, cfg.NSH_PAD], BF16, tag="zmax")
            h_shA = zpool.tile([128, cfg.NSH_PAD], BF16, tag="hshA")
            h_shB = zpool.tile([128, cfg.NSH_PAD], BF16, tag="hshB")
            h_bufs = [h_shA, h_shB]

            def rows_of(t):
                return 128 if t < NT - 1 else last_rows

            def l2norm_bf(pre_t, out_ap):
                """l2-normalize pre_t [128,H] f32 -> bf16 out_ap."""
                sq = wpool.tile([128, H], F32, tag="sq")
                ss = wpool.tile([128, 1], F32, tag="ss")
                nc.scalar.activation(sq[:], pre_t[:], AF.Square, accum_out=ss[:])
                s1 = wpool.tile([128, 1], F32, tag="s1")
                nc.scalar.activation(s1[:], ss[:], AF.Sqrt)
                s2 = wpool.tile([128, 1], F32, tag="s2")
                nc.vector.tensor_scalar_max(s2[:], s1[:], EPS)
                rec = wpool.tile([128, 1], F32, tag="rec")
                nc.vector.reciprocal(rec[:], s2[:])
                nc.vector.tensor_scalar_mul(out_ap, pre_t[:], rec[:])

            # ================= EMBED =================
            for t in range(NT):
                ps = ppool.tile([128, H], F32, tag="acc")
                nc.tensor.matmul(ps[:], ones_t[:], bemb_t[:], start=True, stop=False)
                nc.tensor.matmul(ps[:], xT_t[:, t * 128:(t + 1) * 128], wemb_t[:],
                                 start=False, stop=True)
                h0f = wpool.tile([128, H], F32, tag="pref")
                nc.scalar.activation(h0f[:], ps[:], AF.Relu)
                hsl = h_bufs[0][:, t * 128:(t + 1) * 128]
                l2norm_bf(h0f, hsl)
                nc.vector.tensor_copy(zmax_t[:, t * 128:(t + 1) * 128], hsl)
                r = rows_of(t)
                nc.sync.dma_start(out=shard_out[t * 128: t * 128 + r, :],
                                  in_=h_bufs[0][:r, t * 128: t * 128 + H])
            nc.gpsimd.collective_compute(
                "AllGather", ALU.bypass, replica_groups=RG,
                ins=[shard_out[:]], outs=[tabA[:]],
            )

            # ================= GRU LAYERS =================
            for l in range(L):
                tab_prev = tabA if l % 2 == 0 else tabB
                tab_cur = tabB if l % 2 == 0 else tabA
                for g_, ts in enumerate(groups):
                    lo_off, lo_n, hi_off, hi_n = call_off[g_]
                    glo = gpool.tile([128, lo_n * 128], BF16, tag="glo")
                    ghi = gpool.tile([128, hi_n * 128], BF16, tag="ghi")
                    nc.gpsimd.dma_gather(
                        glo[:].rearrange("p (c e) -> p c e", e=128),
                        tab_prev[0:cfg.SPLIT, :],
                        gidx_t[:, lo_off: lo_off + 8 * lo_n],
                        lo_n * 128, lo_n * 128, H,
                        single_packet=cfg.single_packet,
                        queue_num=(2 * g_) % 4,
                    )
                    nc.gpsimd.dma_gather(
                        ghi[:].rearrange("p (c e) -> p c e", e=128),
                        tab_prev[cfg.SPLIT: cfg.N, :],
                        gidx_t[:, hi_off: hi_off + 8 * hi_n],
                        hi_n * 128, hi_n * 128, H,
                        single_packet=cfg.single_packet,
                        queue_num=(2 * g_ + 1) % 4,
                    )
                    for t in ts:
                        r = rows_of(t)
                        base = t * 128
                        h_prev = h_bufs[l % 2]
                        h_next = h_bufs[(l + 1) % 2]
                        hp_sl = h_prev[:, base:base + H]
                        tps = tpool.tile([128, H], BF16, tag="tps")
                        nc.tensor.transpose(tps[:], hp_sl, ident_t[:])
                        hT = wpool.tile([128, H], BF16, tag="hT")
                        nc.scalar.activation(hT[:], tps[:], AF.Copy)

                        # ---- wide one-hot S for all chunks of this tile ----
                        nch = int(nlo[t] + nhi[t])
                        cid0 = chunk_id[(t, 0, 0)]
                        S = spool.tile([128, nch * 128], BF16, tag="S")
                        S3 = S[:].rearrange("p (c e) -> p c e", e=128)
                        iota_b = iota_t[:].unsqueeze(1).broadcast_to((128, nch, 128))
                        col_b = cmeta_t[:, cid0:cid0 + nch].unsqueeze(2) \
                            .broadcast_to((128, nch, 128))
                        nrm_b = cmeta_t[:, NCHUNK + cid0:NCHUNK + cid0 + nch] \
                            .unsqueeze(2).broadcast_to((128, nch, 128))
                        nc.vector.tensor_tensor(S3, iota_b, col_b, ALU.is_equal)
                        nc.vector.tensor_tensor(S3, S3, nrm_b, ALU.mult)

                        # ---- scatter: aggT[H, v] += G_chunk^T @ S_chunk ----
                        aggT_ps = ppool.tile([128, H], F32, tag="acc")
                        for k in range(nch):
                            if k < int(nlo[t]):
                                buf, bp = glo, buf_pos[(t, 0, k)]
                            else:
                                buf, bp = ghi, buf_pos[(t, 1, k - int(nlo[t]))]
                            nc.tensor.matmul(
                                aggT_ps[:],
                                buf[:, bp * 128:(bp + 1) * 128],
                                S[:, k * 128:(k + 1) * 128],
                                start=(k == 0), stop=(k == nch - 1),
                            )
                        aggT = wpool.tile([128, H], BF16, tag="aggT")
                        nc.scalar.activation(aggT[:], aggT_ps[:], AF.Copy)

                        # ---- GRU gate matmuls (fused bias openers) ----
                        grz = qpool.tile([128, 256], F32, tag="grz")
                        gn = qpool.tile([128, 256], F32, tag="gn")
                        nc.tensor.matmul(grz[:], ones_t[:],
                                         ball_t[:, l * 512:l * 512 + 256],
                                         start=True, stop=False)
                        nc.tensor.matmul(grz[:], aggT[:],
                                         wih_rz_t[:, l * 256:(l + 1) * 256],
                                         start=False, stop=False)
                        nc.tensor.matmul(grz[:], hT[:],
                                         whh_rz_t[:, l * 256:(l + 1) * 256],
                                         start=False, stop=True)
                        nc.tensor.matmul(gn[:, 0:128], ones_t[:],
                                         ball_t[:, l * 512 + 256:l * 512 + 384],
                                         start=True, stop=False)
                        nc.tensor.matmul(gn[:, 0:128], aggT[:],
                                         wih_n_t[:, l * 128:(l + 1) * 128],
                                         start=False, stop=True)
                        nc.tensor.matmul(gn[:, 128:256], ones_t[:],
                                         ball_t[:, l * 512 + 384:(l + 1) * 512],
                                         start=True, stop=False)
                        nc.tensor.matmul(gn[:, 128:256], hT[:],
                                         whh_n_t[:, l * 128:(l + 1) * 128],
                                         start=False, stop=True)

                        # ---- gates ----
                        rzt = wpool.tile([128, 256], F32, tag="rzt")
                        nc.scalar.activation(rzt[:], grz[:], AF.Sigmoid)
                        t1 = wpool.tile([128, H], F32, tag="t1")
                        nc.vector.tensor_mul(t1[:], rzt[:, 0:128], gn[:, 128:256])
                        t2 = wpool.tile([128, H], F32, tag="t2")
                        nc.vector.tensor_add(t2[:], t1[:], gn[:, 0:128])
                        ng = wpool.tile([128, H], F32, tag="ng")
                        nc.scalar.activation(ng[:], t2[:], AF.Tanh)
                        d = wpool.tile([128, H], F32, tag="d")
                        nc.vector.tensor_sub(d[:], hp_sl, ng[:])
                        e = wpool.tile([128, H], F32, tag="e")
                        nc.vector.tensor_mul(e[:], d[:], rzt[:, 128:256])
                        pre = wpool.tile([128, H], F32, tag="pref")
                        nc.vector.tensor_add(pre[:], e[:], ng[:])
                        hn_sl = h_next[:, base:base + H]
                        l2norm_bf(pre, hn_sl)
                        nc.vector.tensor_max(zmax_t[:, base:base + 128],
                                             zmax_t[:, base:base + 128], hn_sl)
                        if l < L - 1:
                            nc.sync.dma_start(out=shard_out[base: base + r, :],
                                              in_=h_next[:r, base:base + H])
                if l < L - 1:
                    nc.gpsimd.collective_compute(
                        "AllGather", ALU.bypass, replica_groups=RG,
                        ins=[shard_out[:]], outs=[tab_cur[:]],
                    )

            # ================= DECODER =================
            for t in range(NT):
                r = rows_of(t)
                base = t * 128
                tps = tpool.tile([128, H], BF16, tag="tps")
                nc.tensor.transpose(tps[:], zmax_t[:, base:base + 128], ident_t[:])
                zT = wpool.tile([128, H], BF16, tag="zT")
                nc.scalar.activation(zT[:], tps[:], AF.Copy)
                hid_ps = qpool.tile([cfg.HID, 128], F32, tag="grz")
                nc.tensor.matmul(hid_ps[:], whid_t[:], zT[:], start=True, stop=True)
                hid = wpool.tile([cfg.HID, 128], BF16, tag="hid")
                nc.scalar.activation(hid[:], hid_ps[:], AF.Relu, bias=bhid_t[:])
                o_ps = ppool.tile([1, 128], F32, tag="acc")
                nc.tensor.matmul(o_ps[:], wout_t[:], hid[:], start=True, stop=True)
                o_sb = wpool.tile([1, 128], F32, tag="osb")
                nc.scalar.activation(o_sb[:], o_ps[:], AF.Copy, bias=float(b_out_val))
                nc.sync.dma_start(out=p_out[base: base + r, :], in_=o_sb[:1, :r])
    nc.compile()
    return nc


def make_in_maps(cfg, inputs, plan, gidx, cmeta):
    bf = ml_dtypes.bfloat16
    L, H, NSH = cfg.L, cfg.H, cfg.NSH
    x = np.asarray(inputs["x"], np.float32)
    w_ih = np.asarray(inputs["w_ih"], np.float32)
    w_hh = np.asarray(inputs["w_hh"], np.float32)
    b_ih = np.asarray(inputs["b_ih"], np.float32)
    b_hh = np.asarray(inputs["b_hh"], np.float32)

    wih_rz = np.concatenate([w_ih[l, :256, :].T for l in range(L)], axis=1)
    wih_n = np.concatenate([w_ih[l, 256:384, :].T for l in range(L)], axis=1)
    whh_rz = np.concatenate([w_hh[l, :256, :].T for l in range(L)], axis=1)
    whh_n = np.concatenate([w_hh[l, 256:384, :].T for l in range(L)], axis=1)
    ball = np.concatenate(
        [np.concatenate([b_ih[l, :256] + b_hh[l, :256],
                         b_ih[l, 256:384], b_hh[l, 256:384]])
         for l in range(L)])[None, :]

    common = {
        "iota": np.tile(np.arange(128, dtype=np.float32), (128, 1)),
        "ident": np.eye(128, dtype=bf),
        "wembT": np.ascontiguousarray(np.asarray(inputs["W_embed"], np.float32).T),
        "bemb": np.asarray(inputs["b_embed"], np.float32)[None, :],
        "wih_rz": np.ascontiguousarray(wih_rz, dtype=bf),
        "wih_n": np.ascontiguousarray(wih_n, dtype=bf),
        "whh_rz": np.ascontiguousarray(whh_rz, dtype=bf),
        "whh_n": np.ascontiguousarray(whh_n, dtype=bf),
        "ball": np.ascontiguousarray(ball),
        "whidT": np.ascontiguousarray(np.asarray(inputs["W_hid"], np.float32).T,
                                      dtype=bf),
        "bhid": np.asarray(inputs["b_hid"], np.float32)[:, None],
        "woutT": np.ascontiguousarray(np.asarray(inputs["W_out"], np.float32).T,
                                      dtype=bf),
        "ones1": np.ones((1, 128), np.float32),
    }
    in_maps = []
    for c in range(cfg.n_cores):
        xT = np.zeros((cfg.IN, cfg.NSH_PAD), np.float32)
        xT[:, :NSH] = x[c * NSH:(c + 1) * NSH, :].T
        m = dict(common)
        m["xT"] = xT
        m["gidx"] = gidx[c]
        m["cmeta"] = cmeta[c]
        in_maps.append(m)
    return in_maps


def kernel(**inputs):
    cfg = Cfg()
    plan, gidx, cmeta = build_plan(cfg, np.asarray(inputs["edge_idx"]))
    nc = build_nc(cfg, plan, float(np.asarray(inputs["b_out"]).ravel()[0]))
    in_maps = make_in_maps(cfg, inputs, plan, gidx, cmeta)
    res = run_bass_kernel_spmd(nc, in_maps, list(range(cfg.n_cores)))
    out = np.concatenate([res.results[c]["out"] for c in range(cfg.n_cores)], axis=0)
    return out.astype(np.float32)



# revision 14
# speedup vs baseline: 1.5709x; 1.1601x over previous
"""DrBC GNN message-passing kernel for 8 Trainium2 NeuronCores.

Sharding: nodes split contiguously across 8 cores. Edges bucketed by target
node's owner core and 128-node target tile. Per layer each core:
  1. dma_gather's source-node rows (bf16) from a replicated node table in HBM
     (split into lo/hi halves so indices fit int16),
  2. scatter-adds them into per-tile aggregates via PE matmuls against
     on-the-fly one-hot selection matrices (edge norm folded in),
  3. runs the GRU update (bias-opener matmuls + fused gi+gh accumulation),
  4. l2-normalizes, updates the running layer-max, and
  5. AllGathers the new shard into the next layer's replicated table
     (skipped after the last layer).
Decoder (2-layer MLP on the layer-max) runs on the local shard only.
"""

import numpy as np
import ml_dtypes

import concourse.bass as bass
import concourse.bacc as bacc
import concourse.mybir as mybir
from concourse.tile import TileContext
from concourse.bass_utils import run_bass_kernel_spmd

F32 = mybir.dt.float32
BF16 = mybir.dt.bfloat16
I16 = mybir.dt.int16
AF = mybir.ActivationFunctionType
ALU = mybir.AluOpType

EPS = 1e-12


class Cfg:
    def __init__(self, N=50000, E=600000, L=5, n_cores=8, group_tiles=2,
                 single_packet=False):
        self.N, self.E, self.L, self.n_cores = N, E, L, n_cores
        self.H = 128
        self.IN = 3
        self.HID = 64
        assert N % n_cores == 0
        self.NSH = N // n_cores              # nodes per core
        self.NT = (self.NSH + 127) // 128    # node tiles per core
        self.NSH_PAD = self.NT * 128
        self.SPLIT = (N // 2 + 127) // 128 * 128  # lo/hi table split (int16 idx)
        assert self.SPLIT < 32768 and N - self.SPLIT < 32768
        self.GT = group_tiles                # tiles per gather group
        self.single_packet = single_packet


def build_plan(cfg, edge_idx):
    """Bucket edges by (core, target tile), pad chunk counts uniformly across
    cores (SPMD: one NEFF), build per-core gather-index/chunk-meta arrays."""
    row = np.asarray(edge_idx[0], dtype=np.int64)
    col = np.asarray(edge_idx[1], dtype=np.int64)
    N = cfg.N
    deg = np.bincount(col, minlength=N).astype(np.float64) + 1.0
    dinv = deg ** -0.5
    norm = (dinv[row] * dinv[col]).astype(np.float32)

    n_cores, NT = cfg.n_cores, cfg.NT
    core_of = col // cfg.NSH
    tile_of = (col % cfg.NSH) // 128
    v_of = ((col % cfg.NSH) % 128).astype(np.float32)
    is_hi = (row >= cfg.SPLIT).astype(np.int64)
    rloc = np.where(is_hi == 1, row - cfg.SPLIT, row).astype(np.int16)

    # bucket key: (core, tile, half)
    key = (core_of * NT + tile_of) * 2 + is_hi
    nkey = n_cores * NT * 2
    order = np.argsort(key, kind="stable")
    ks, rs, vs, ns = key[order], rloc[order], v_of[order], norm[order]
    counts = np.bincount(ks, minlength=nkey)
    starts = np.concatenate([[0], np.cumsum(counts)[:-1]])

    cnt = counts.reshape(n_cores, NT, 2)
    # uniform per-tile chunk counts = max over cores (>=1 so psum groups exist)
    nlo = np.maximum((cnt[:, :, 0].max(axis=0) + 127) // 128, 1)
    nhi = np.maximum((cnt[:, :, 1].max(axis=0) + 127) // 128, 1)
    nch2 = np.stack([nlo, nhi], axis=1)          # [NT, 2]

    groups = [list(range(t0, min(t0 + cfg.GT, NT))) for t0 in range(0, NT, cfg.GT)]
    NCHUNK = int(nlo.sum() + nhi.sum())
    W_tot = 8 * NCHUNK

    # padded flat layout, order (t, half) shared by all cores
    pad_off = np.zeros((NT, 2), dtype=np.int64)
    po = 0
    for t in range(NT):
        for h in (0, 1):
            pad_off[t, h] = po
            po += int(nch2[t, h]) * 128
    PADTOT = po

    # scatter edges into per-core padded arrays
    idxpad = np.zeros((n_cores, PADTOT), dtype=np.int16)
    vpad = np.zeros((n_cores, PADTOT), dtype=np.float32)
    npad = np.zeros((n_cores, PADTOT), dtype=np.float32)
    rank = np.arange(len(ks)) - starts[ks]
    kc, kt, kh = ks // (NT * 2), (ks // 2) % NT, ks % 2
    dest = pad_off[kt, kh] + rank
    idxpad[kc, dest] = rs
    vpad[kc, dest] = vs
    npad[kc, dest] = ns

    # chunk ids in emission order: per group, per tile: lo chunks then hi chunks
    chunk_id = {}
    cid = 0
    for ts in groups:
        for t in ts:
            for j in range(int(nlo[t])):
                chunk_id[(t, 0, j)] = cid; cid += 1
            for j in range(int(nhi[t])):
                chunk_id[(t, 1, j)] = cid; cid += 1
    assert cid == NCHUNK

    # gather-call layout: per group, one lo call then one hi call
    call_off = []
    buf_pos = {}
    col_cursor = 0
    for ts in groups:
        lo_n = int(sum(nlo[t] for t in ts))
        hi_n = int(sum(nhi[t] for t in ts))
        call_off.append((col_cursor, lo_n, col_cursor + 8 * lo_n, hi_n))
        col_cursor += 8 * (lo_n + hi_n)
        p = 0
        for t in ts:
            for j in range(int(nlo[t])):
                buf_pos[(t, 0, j)] = p; p += 1
        p = 0
        for t in ts:
            for j in range(int(nhi[t])):
                buf_pos[(t, 1, j)] = p; p += 1
    assert col_cursor == W_tot

    gidx = np.zeros((n_cores, 128, W_tot), dtype=np.int16)
    cmeta = np.zeros((n_cores, 128, 2 * NCHUNK), dtype=np.float32)
    for c in range(n_cores):
        for g_, ts in enumerate(groups):
            lo_off, lo_n, hi_off, hi_n = call_off[g_]
            for half, off, nch in ((0, lo_off, nlo), (1, hi_off, nhi)):
                parts = [idxpad[c, pad_off[t, half]:
                                pad_off[t, half] + int(nch[t]) * 128]
                         for t in ts]
                idx_flat = np.concatenate(parts)
                wrapped = idx_flat.reshape(-1, 16).T
                gidx[c, :, off:off + wrapped.shape[1]] = np.tile(wrapped, (8, 1))
        for t in range(NT):
            for half, nch in ((0, nlo), (1, nhi)):
                c0 = chunk_id[(t, half, 0)]
                n_ = int(nch[t])
                sl = slice(pad_off[t, half], pad_off[t, half] + n_ * 128)
                cmeta[c, :, c0:c0 + n_] = vpad[c, sl].reshape(n_, 128).T
                cmeta[c, :, NCHUNK + c0:NCHUNK + c0 + n_] = \
                    npad[c, sl].reshape(n_, 128).T

    # host-precomputed one-hot scatter matrices: S[p, cid*128 + v] = norm of the
    # edge in slot p of chunk cid targeting within-tile node v (0 for pad slots)
    bf = ml_dtypes.bfloat16
    S_full = np.zeros((n_cores, 128, NCHUNK * 128), dtype=bf)
    for c in range(n_cores):
        for t in range(NT):
            for half, nch_arr in ((0, nlo), (1, nhi)):
                c0 = chunk_id[(t, half, 0)]
                n_ = int(nch_arr[t])
                sl = slice(pad_off[t, half], pad_off[t, half] + n_ * 128)
                v = vpad[c, sl].astype(np.int64).reshape(n_, 128)   # [chunk, p]
                nval = npad[c, sl].reshape(n_, 128)
                for j in range(n_):
                    S_full[c, np.arange(128), (c0 + j) * 128 + v[j]] = \
                        nval[j].astype(bf)

    return dict(nlo=nlo, nhi=nhi, groups=groups, NCHUNK=NCHUNK, W_tot=W_tot,
                call_off=call_off, chunk_id=chunk_id, buf_pos=buf_pos,
                S_full=S_full), gidx, cmeta


def build_nc(cfg, plan, b_out_val):
    nc = bacc.Bacc("TRN2", target_bir_lowering=False, debug=False,
                   num_devices=cfg.n_cores, num_swdge_queues=4,
                   dynamic_dma_scratch_size=32768)
    H, NT, NSH, L = cfg.H, cfg.NT, cfg.NSH, cfg.L
    NCHUNK = plan["NCHUNK"]
    nlo, nhi, groups = plan["nlo"], plan["nhi"], plan["groups"]
    call_off, chunk_id, buf_pos = plan["call_off"], plan["chunk_id"], plan["buf_pos"]
    RG = [list(range(cfg.n_cores))]
    last_rows = NSH - (NT - 1) * 128

    p_gidx = nc.declare_dram_parameter("gidx", [128, plan["W_tot"]], I16, isOutput=False)
    p_S = nc.declare_dram_parameter("S_full", [128, NCHUNK * 128], BF16, isOutput=False)
    p_xT = nc.declare_dram_parameter("xT", [cfg.IN, cfg.NSH_PAD], F32, isOutput=False)
    p_ident = nc.declare_dram_parameter("ident", [128, 128], BF16, isOutput=False)
    p_wemb = nc.declare_dram_parameter("wembT", [cfg.IN, H], F32, isOutput=False)
    p_bemb = nc.declare_dram_parameter("bemb", [1, H], F32, isOutput=False)
    p_wih_rz = nc.declare_dram_parameter("wih_rz", [H, L * 256], BF16, isOutput=False)
    p_wih_n = nc.declare_dram_parameter("wih_n", [H, L * 128], BF16, isOutput=False)
    p_whh_rz = nc.declare_dram_parameter("whh_rz", [H, L * 256], BF16, isOutput=False)
    p_whh_n = nc.declare_dram_parameter("whh_n", [H, L * 128], BF16, isOutput=False)
    p_ball = nc.declare_dram_parameter("ball", [1, L * 512], F32, isOutput=False)
    p_whid = nc.declare_dram_parameter("whidT", [H, cfg.HID], BF16, isOutput=False)
    p_bhid = nc.declare_dram_parameter("bhid", [cfg.HID, 1], F32, isOutput=False)
    p_wout = nc.declare_dram_parameter("woutT", [cfg.HID, 1], BF16, isOutput=False)
    p_ones = nc.declare_dram_parameter("ones1", [1, 128], F32, isOutput=False)
    p_out = nc.declare_dram_parameter("out", [NSH, 1], F32, isOutput=True)

    tabA = nc.dram_tensor("tabA", [cfg.N, H], BF16, addr_space="Shared")
    tabB = nc.dram_tensor("tabB", [cfg.N, H], BF16, addr_space="Shared")
    shard_out = nc.dram_tensor("shard_out", [NSH, H], BF16)

    with TileContext(nc) as tc:
        with (
            tc.tile_pool(name="consts", bufs=1) as cpool,
            tc.tile_pool(name="zpool", bufs=1) as zpool,
            tc.tile_pool(name="gbuf", bufs=2) as gpool,
            tc.tile_pool(name="work", bufs=3) as wpool,
            tc.tile_pool(name="spool", bufs=2) as spool,
            tc.tile_pool(name="psA", bufs=2, space="PSUM") as ppool,
            tc.tile_pool(name="psB", bufs=2, space="PSUM") as qpool,
            tc.tile_pool(name="psT", bufs=2, space="PSUM") as tpool,
        ):
            def load_const(ap, dtype, tag):
                t = cpool.tile(list(ap.shape), dtype, tag=tag)
                nc.sync.dma_start(out=t[:], in_=ap[:])
                return t

            gidx_t = load_const(p_gidx, I16, "gidx")
            xT_t = load_const(p_xT, F32, "xT")
            ident_t = load_const(p_ident, BF16, "ident")
            wemb_t = load_const(p_wemb, F32, "wemb")
            bemb_t = load_const(p_bemb, F32, "bemb")
            wih_rz_t = load_const(p_wih_rz, BF16, "wihrz")
            wih_n_t = load_const(p_wih_n, BF16, "wihn")
            whh_rz_t = load_const(p_whh_rz, BF16, "whhrz")
            whh_n_t = load_const(p_whh_n, BF16, "whhn")
            ball_t = load_const(p_ball, F32, "ball")
            whid_t = load_const(p_whid, BF16, "whid")
            bhid_t = load_const(p_bhid, F32, "bhid")
            wout_t = load_const(p_wout, BF16, "wout")
            ones_t = load_const(p_ones, F32, "ones")

            zmax_t = zpool.tile([128, cfg.NSH_PAD], BF16, tag="zmax")
            h_shA = zpool.tile([128, cfg.NSH_PAD], BF16, tag="hshA")
            h_shB = zpool.tile([128, cfg.NSH_PAD], BF16, tag="hshB")
            h_bufs = [h_shA, h_shB]

            def rows_of(t):
                return 128 if t < NT - 1 else last_rows

            def l2norm_bf(pre_t, out_ap):
                """l2-normalize pre_t [128,H] f32 -> bf16 out_ap."""
                sq = wpool.tile([128, H], F32, tag="sq")
                ss = wpool.tile([128, 1], F32, tag="ss")
                nc.scalar.activation(sq[:], pre_t[:], AF.Square, accum_out=ss[:])
                s1 = wpool.tile([128, 1], F32, tag="s1")
                nc.scalar.activation(s1[:], ss[:], AF.Sqrt)
                s2 = wpool.tile([128, 1], F32, tag="s2")
                nc.vector.tensor_scalar_max(s2[:], s1[:], EPS)
                rec = wpool.tile([128, 1], F32, tag="rec")
                nc.vector.reciprocal(rec[:], s2[:])
                nc.vector.tensor_tensor(out_ap, pre_t[:],
                                        rec[:].broadcast_to((128, H)), ALU.mult)

            # ================= EMBED =================
            for t in range(NT):
                ps = ppool.tile([128, H], F32, tag="acc")
                nc.tensor.matmul(ps[:], ones_t[:], bemb_t[:], start=True, stop=False)
                nc.tensor.matmul(ps[:], xT_t[:, t * 128:(t + 1) * 128], wemb_t[:],
                                 start=False, stop=True)
                h0f = wpool.tile([128, H], F32, tag="pref")
                nc.scalar.activation(h0f[:], ps[:], AF.Relu)
                hsl = h_bufs[0][:, t * 128:(t + 1) * 128]
                l2norm_bf(h0f, hsl)
                nc.vector.tensor_copy(zmax_t[:, t * 128:(t + 1) * 128], hsl)
                r = rows_of(t)
                nc.sync.dma_start(out=shard_out[t * 128: t * 128 + r, :],
                                  in_=h_bufs[0][:r, t * 128: t * 128 + H])
            nc.gpsimd.collective_compute(
                "AllGather", ALU.bypass, replica_groups=RG,
                ins=[shard_out[:]], outs=[tabA[:]],
            )

            # ================= GRU LAYERS =================
            for l in range(L):
                tab_prev = tabA if l % 2 == 0 else tabB
                tab_cur = tabB if l % 2 == 0 else tabA
                for g_, ts in enumerate(groups):
                    lo_off, lo_n, hi_off, hi_n = call_off[g_]
                    cid_g0 = chunk_id[(ts[0], 0, 0)]
                    gch = int(sum(nlo[t] + nhi[t] for t in ts))
                    Sg = spool.tile([128, gch * 128], BF16, tag="S")
                    nc.sync.dma_start(
                        out=Sg[:],
                        in_=p_S[:, cid_g0 * 128:(cid_g0 + gch) * 128])
                    glo = gpool.tile([128, lo_n * 128], BF16, tag="glo")
                    ghi = gpool.tile([128, hi_n * 128], BF16, tag="ghi")
                    nc.gpsimd.dma_gather(
                        glo[:].rearrange("p (c e) -> p c e", e=128),
                        tab_prev[0:cfg.SPLIT, :],
                        gidx_t[:, lo_off: lo_off + 8 * lo_n],
                        lo_n * 128, lo_n * 128, H,
                        single_packet=cfg.single_packet,
                        queue_num=(2 * g_) % 4,
                    )
                    nc.gpsimd.dma_gather(
                        ghi[:].rearrange("p (c e) -> p c e", e=128),
                        tab_prev[cfg.SPLIT: cfg.N, :],
                        gidx_t[:, hi_off: hi_off + 8 * hi_n],
                        hi_n * 128, hi_n * 128, H,
                        single_packet=cfg.single_packet,
                        queue_num=(2 * g_ + 1) % 4,
                    )
                    for t in ts:
                        r = rows_of(t)
                        base = t * 128
                        h_prev = h_bufs[l % 2]
                        h_next = h_bufs[(l + 1) % 2]
                        hp_sl = h_prev[:, base:base + H]
                        tps = tpool.tile([128, H], BF16, tag="tps")
                        nc.tensor.transpose(tps[:], hp_sl, ident_t[:])
                        hT = wpool.tile([128, H], BF16, tag="hT")
                        nc.scalar.activation(hT[:], tps[:], AF.Copy)

                        # ---- host-precomputed one-hot S slice for this tile ----
                        nch = int(nlo[t] + nhi[t])
                        cid0 = chunk_id[(t, 0, 0)]
                        sbase = cid0 - cid_g0

                        # ---- scatter: aggT[H, v] += G_chunk^T @ S_chunk ----
                        aggT_ps = ppool.tile([128, H], F32, tag="acc")
                        for k in range(nch):
                            if k < int(nlo[t]):
                                buf, bp = glo, buf_pos[(t, 0, k)]
                            else:
                                buf, bp = ghi, buf_pos[(t, 1, k - int(nlo[t]))]
                            nc.tensor.matmul(
                                aggT_ps[:],
                                buf[:, bp * 128:(bp + 1) * 128],
                                Sg[:, (sbase + k) * 128:(sbase + k + 1) * 128],
                                start=(k == 0), stop=(k == nch - 1),
                            )
                        aggT = wpool.tile([128, H], BF16, tag="aggT")
                        nc.scalar.activation(aggT[:], aggT_ps[:], AF.Copy)

                        # ---- GRU gate matmuls (fused bias openers) ----
                        grz = qpool.tile([128, 256], F32, tag="grz")
                        gn = qpool.tile([128, 256], F32, tag="gn")
                        nc.tensor.matmul(grz[:], ones_t[:],
                                         ball_t[:, l * 512:l * 512 + 256],
                                         start=True, stop=False)
                        nc.tensor.matmul(grz[:], aggT[:],
                                         wih_rz_t[:, l * 256:(l + 1) * 256],
                                         start=False, stop=False)
                        nc.tensor.matmul(grz[:], hT[:],
                                         whh_rz_t[:, l * 256:(l + 1) * 256],
                                         start=False, stop=True)
                        nc.tensor.matmul(gn[:, 0:128], ones_t[:],
                                         ball_t[:, l * 512 + 256:l * 512 + 384],
                                         start=True, stop=False)
                        nc.tensor.matmul(gn[:, 0:128], aggT[:],
                                         wih_n_t[:, l * 128:(l + 1) * 128],
                                         start=False, stop=True)
                        nc.tensor.matmul(gn[:, 128:256], ones_t[:],
                                         ball_t[:, l * 512 + 384:(l + 1) * 512],
                                         start=True, stop=False)
                        nc.tensor.matmul(gn[:, 128:256], hT[:],
                                         whh_n_t[:, l * 128:(l + 1) * 128],
                                         start=False, stop=True)

                        # ---- gates ----
                        rzt = wpool.tile([128, 256], F32, tag="rzt")
                        nc.scalar.activation(rzt[:], grz[:], AF.Sigmoid)
                        t1 = wpool.tile([128, H], F32, tag="t1")
                        nc.vector.tensor_mul(t1[:], rzt[:, 0:128], gn[:, 128:256])
                        t2 = wpool.tile([128, H], F32, tag="t2")
                        nc.vector.tensor_add(t2[:], t1[:], gn[:, 0:128])
                        ng = wpool.tile([128, H], F32, tag="ng")
                        nc.scalar.activation(ng[:], t2[:], AF.Tanh)
                        d = wpool.tile([128, H], F32, tag="d")
                        nc.vector.tensor_sub(d[:], hp_sl, ng[:])
                        e = wpool.tile([128, H], F32, tag="e")
                        nc.vector.tensor_mul(e[:], d[:], rzt[:, 128:256])
                        pre = wpool.tile([128, H], F32, tag="pref")
                        nc.vector.tensor_add(pre[:], e[:], ng[:])
                        hn_sl = h_next[:, base:base + H]
                        l2norm_bf(pre, hn_sl)
                        nc.vector.tensor_max(zmax_t[:, base:base + 128],
                                             zmax_t[:, base:base + 128], hn_sl)
                        if l < L - 1:
                            nc.sync.dma_start(out=shard_out[base: base + r, :],
                                              in_=h_next[:r, base:base + H])
                if l < L - 1:
                    nc.gpsimd.collective_compute(
                        "AllGather", ALU.bypass, replica_groups=RG,
                        ins=[shard_out[:]], outs=[tab_cur[:]],
                    )

            # ================= DECODER =================
            for t in range(NT):
                r = rows_of(t)
                base = t * 128
                tps = tpool.tile([128, H], BF16, tag="tps")
                nc.tensor.transpose(tps[:], zmax_t[:, base:base + 128], ident_t[:])
                zT = wpool.tile([128, H], BF16, tag="zT")
                nc.scalar.activation(zT[:], tps[:], AF.Copy)
                hid_ps = qpool.tile([cfg.HID, 128], F32, tag="grz")
                nc.tensor.matmul(hid_ps[:], whid_t[:], zT[:], start=True, stop=True)
                hid = wpool.tile([cfg.HID, 128], BF16, tag="hid")
                nc.scalar.activation(hid[:], hid_ps[:], AF.Relu, bias=bhid_t[:])
                o_ps = ppool.tile([1, 128], F32, tag="acc")
                nc.tensor.matmul(o_ps[:], wout_t[:], hid[:], start=True, stop=True)
                o_sb = wpool.tile([1, 128], F32, tag="osb")
                nc.scalar.activation(o_sb[:], o_ps[:], AF.Copy, bias=float(b_out_val))
                nc.sync.dma_start(out=p_out[base: base + r, :], in_=o_sb[:1, :r])
    nc.compile()
    return nc


def make_in_maps(cfg, inputs, plan, gidx, cmeta):
    bf = ml_dtypes.bfloat16
    L, H, NSH = cfg.L, cfg.H, cfg.NSH
    x = np.asarray(inputs["x"], np.float32)
    w_ih = np.asarray(inputs["w_ih"], np.float32)
    w_hh = np.asarray(inputs["w_hh"], np.float32)
    b_ih = np.asarray(inputs["b_ih"], np.float32)
    b_hh = np.asarray(inputs["b_hh"], np.float32)

    wih_rz = np.concatenate([w_ih[l, :256, :].T for l in range(L)], axis=1)
    wih_n = np.concatenate([w_ih[l, 256:384, :].T for l in range(L)], axis=1)
    whh_rz = np.concatenate([w_hh[l, :256, :].T for l in range(L)], axis=1)
    whh_n = np.concatenate([w_hh[l, 256:384, :].T for l in range(L)], axis=1)
    ball = np.concatenate(
        [np.concatenate([b_ih[l, :256] + b_hh[l, :256],
                         b_ih[l, 256:384], b_hh[l, 256:384]])
         for l in range(L)])[None, :]

    common = {
        "ident": np.eye(128, dtype=bf),
        "wembT": np.ascontiguousarray(np.asarray(inputs["W_embed"], np.float32).T),
        "bemb": np.asarray(inputs["b_embed"], np.float32)[None, :],
        "wih_rz": np.ascontiguousarray(wih_rz, dtype=bf),
        "wih_n": np.ascontiguousarray(wih_n, dtype=bf),
        "whh_rz": np.ascontiguousarray(whh_rz, dtype=bf),
        "whh_n": np.ascontiguousarray(whh_n, dtype=bf),
        "ball": np.ascontiguousarray(ball),
        "whidT": np.ascontiguousarray(np.asarray(inputs["W_hid"], np.float32).T,
                                      dtype=bf),
        "bhid": np.asarray(inputs["b_hid"], np.float32)[:, None],
        "woutT": np.ascontiguousarray(np.asarray(inputs["W_out"], np.float32).T,
                                      dtype=bf),
        "ones1": np.ones((1, 128), np.float32),
    }
    in_maps = []
    for c in range(cfg.n_cores):
        xT = np.zeros((cfg.IN, cfg.NSH_PAD), np.float32)
        xT[:, :NSH] = x[c * NSH:(c + 1) * NSH, :].T
        m = dict(common)
        m["xT"] = xT
        m["gidx"] = gidx[c]
        m["S_full"] = plan["S_full"][c]
        in_maps.append(m)
    return in_maps


def kernel(**inputs):
    cfg = Cfg()
    plan, gidx, cmeta = build_plan(cfg, np.asarray(inputs["edge_idx"]))
    nc = build_nc(cfg, plan, float(np.asarray(inputs["b_out"]).ravel()[0]))
    in_maps = make_in_maps(cfg, inputs, plan, gidx, cmeta)
    res = run_bass_kernel_spmd(nc, in_maps, list(range(cfg.n_cores)))
    out = np.concatenate([res.results[c]["out"] for c in range(cfg.n_cores)], axis=0)
    return out.astype(np.float32)

